# revision 2
# baseline (speedup 1.0000x reference)
"""Trainium2 Bass kernel for fused multi-tensor cosine-similarity loss.

Computes 1 - <r,d> / (|r| |d|) over 10 gradient tensors (5 rec + 5 data,
45,675,264 f32 elements per side), data-parallel across 8 NeuronCores.

Strategy (memory-bound, ~45.7 MB HBM traffic per core):
  - Host packs each side into a flat f32 stream, zero-padded to
    8 cores x T tiles x 128 partitions x F columns (zeros don't affect
    dot products or squared norms).
  - Per core, per tile: DMA r,d tiles to SBUF; one DVE
    tensor_tensor_reduce computes r*d and its per-partition row-sum in a
    single pass; two ACT activation(Square, accum_out=...) compute the
    per-partition row-sums of r^2 and d^2 in one pass each.
  - Per-tile partial sums land in [128, T] accumulators, DMA'd out once.
  - Host reduces the 8 x 3 x 128 x T partials in float64 and applies the
    final cosine combine.
"""

import os
import sys

import numpy as np

_REPO = "/opt/trn_rl_repo"
if _REPO not in sys.path:
    sys.path.insert(0, _REPO)

import concourse.bacc as bacc
import concourse.mybir as mybir
from concourse.bass_utils import run_bass_kernel_spmd
from concourse.tile import TileContext

C = 8  # cores
P = 128  # SBUF partitions
F = 2048  # free-dim columns per tile (1 MiB per DMA)
T = 22  # tiles per core
TOTAL = 45_675_264  # elements per side (sum of the 5 tensor sizes)
PADDED = C * T * P * F  # 46,137,344

_REC_KEYS = ("rec_emb", "rec_qkv", "rec_proj", "rec_fc1", "rec_fc2")
_DATA_KEYS = ("data_emb", "data_qkv", "data_proj", "data_fc1", "data_fc2")

_CACHE = {}


def _build():
    nc = bacc.Bacc("TRN2", target_bir_lowering=False, debug=False)
    r = nc.declare_dram_parameter("r", [T, P, F], mybir.dt.float32, isOutput=False)
    d = nc.declare_dram_parameter("d", [T, P, F], mybir.dt.float32, isOutput=False)
    o = nc.declare_dram_parameter("o", [3, P, T], mybir.dt.float32, isOutput=True)

    f32 = mybir.dt.float32
    with TileContext(nc) as tc:
        with (
            tc.tile_pool(name="io", bufs=4) as io,
            tc.tile_pool(name="scr", bufs=2) as scr,
            tc.tile_pool(name="accp", bufs=1) as accp,
        ):
            acc_dot = accp.tile([P, T], f32)
            acc_rr = accp.tile([P, T], f32)
            acc_dd = accp.tile([P, T], f32)
            for t in range(T):
                rt = io.tile([P, F], f32, tag="rt")
                dt = io.tile([P, F], f32, tag="dt")
                nc.sync.dma_start(out=rt[:], in_=r[t])
                nc.scalar.dma_start(out=dt[:], in_=d[t])
                prod = scr.tile([P, F], f32, tag="prod")
                rsq = scr.tile([P, F], f32, tag="rsq")
                dsq = scr.tile([P, F], f32, tag="dsq")
                # out = (rt bypass 1.0) * dt; accum_out = row-sum(out).
                # Native InstTensorScalarPtr — one DVE pass for the dot.
                nc.vector.scalar_tensor_tensor(
                    out=prod[:],
                    in0=rt[:],
                    scalar=1.0,
                    in1=dt[:],
                    op0=mybir.AluOpType.bypass,
                    op1=mybir.AluOpType.mult,
                    accum_out=acc_dot[:, t : t + 1],
                )
                nc.scalar.activation(
                    rsq[:],
                    rt[:],
                    mybir.ActivationFunctionType.Square,
                    accum_out=acc_rr[:, t : t + 1],
                )
                nc.scalar.activation(
                    dsq[:],
                    dt[:],
                    mybir.ActivationFunctionType.Square,
                    accum_out=acc_dd[:, t : t + 1],
                )
            nc.sync.dma_start(out=o[0], in_=acc_dot[:])
            nc.sync.dma_start(out=o[1], in_=acc_rr[:])
            nc.sync.dma_start(out=o[2], in_=acc_dd[:])
    nc.compile()
    return nc


def _get_nc():
    if "nc" not in _CACHE:
        _CACHE["nc"] = _build()
    return _CACHE["nc"]


def _pack(arrays):
    buf = np.zeros(PADDED, dtype=np.float32)
    off = 0
    for a in arrays:
        a = np.asarray(a, dtype=np.float32)
        n = a.size
        buf[off : off + n] = a.reshape(-1)
        off += n
    assert off == TOTAL
    return buf.reshape(C, T, P, F)


def _run(inputs, trace=False):
    rbuf = _pack([inputs[k] for k in _REC_KEYS])
    dbuf = _pack([inputs[k] for k in _DATA_KEYS])
    in_maps = [{"r": rbuf[c], "d": dbuf[c]} for c in range(C)]
    res = run_bass_kernel_spmd(_get_nc(), in_maps, core_ids=list(range(C)), trace=trace)
    tot = np.zeros(3, dtype=np.float64)
    for m in res.results:
        tot += m["o"].reshape(3, -1).astype(np.float64).sum(axis=1)
    sp, rn, dn = tot
    out = 1.0 - sp / (np.sqrt(rn) * np.sqrt(dn))
    return np.array(out, dtype=np.float32), res


def kernel(**inputs):
    out, _ = _run(inputs, trace=False)
    return out


def kernel_traced(**inputs):
    out, res = _run(inputs, trace=True)
    return out, res


# revision 4
# speedup vs baseline: 1.1555x; 1.1555x over previous
"""Trainium2 Bass kernel for fused multi-tensor cosine-similarity loss.

Computes 1 - <r,d> / (|r| |d|) over 10 gradient tensors (5 rec + 5 data,
45,675,264 f32 elements per side), data-parallel across 8 NeuronCores.

Strategy (memory-bound, ~45.7 MB HBM traffic per core):
  - Host packs each side into a flat f32 stream, zero-padded to
    8 cores x T tiles x 128 partitions x F columns (zeros don't affect
    dot products or squared norms).
  - Per core, per tile: DMA r,d tiles to SBUF; one DVE
    tensor_tensor_reduce computes r*d and its per-partition row-sum in a
    single pass; two ACT activation(Square, accum_out=...) compute the
    per-partition row-sums of r^2 and d^2 in one pass each.
  - Per-tile partial sums land in [128, T] accumulators, DMA'd out once.
  - Host reduces the 8 x 3 x 128 x T partials in float64 and applies the
    final cosine combine.
"""

import os
import sys

import numpy as np

_REPO = "/opt/trn_rl_repo"
if _REPO not in sys.path:
    sys.path.insert(0, _REPO)

import concourse.bacc as bacc
import concourse.mybir as mybir
from concourse.bass_utils import run_bass_kernel_spmd
from concourse.tile import TileContext

C = 8  # cores
P = 128  # SBUF partitions
F = 2048  # free-dim columns per tile (1 MiB per DMA)
T = 22  # tiles per core
TOTAL = 45_675_264  # elements per side (sum of the 5 tensor sizes)
PADDED = C * T * P * F  # 46,137,344

_REC_KEYS = ("rec_emb", "rec_qkv", "rec_proj", "rec_fc1", "rec_fc2")
_DATA_KEYS = ("data_emb", "data_qkv", "data_proj", "data_fc1", "data_fc2")

_CACHE = {}


def _build():
    nc = bacc.Bacc("TRN2", target_bir_lowering=False, debug=False)
    r = nc.declare_dram_parameter("r", [T, P, F], mybir.dt.float32, isOutput=False)
    d = nc.declare_dram_parameter("d", [T, P, F], mybir.dt.float32, isOutput=False)
    o = nc.declare_dram_parameter("o", [3, P, T], mybir.dt.float32, isOutput=True)

    f32 = mybir.dt.float32
    with TileContext(nc) as tc:
        with (
            tc.tile_pool(name="io", bufs=6) as io,
            tc.tile_pool(name="scr", bufs=1) as scr,
            tc.tile_pool(name="accp", bufs=1) as accp,
        ):
            acc_dot = accp.tile([P, T], f32)
            acc_rr = accp.tile([P, T], f32)
            acc_dd = accp.tile([P, T], f32)
            for t in range(T):
                rt = io.tile([P, F], f32, tag="rt")
                dt = io.tile([P, F], f32, tag="dt")
                nc.sync.dma_start(out=rt[:], in_=r[t])
                nc.sync.dma_start(out=dt[:], in_=d[t])
                prod = scr.tile([P, F], f32, tag="prod")
                rsq = scr.tile([P, F], f32, tag="rsq")
                dsq = scr.tile([P, F], f32, tag="dsq")
                # out = (rt bypass 1.0) * dt; accum_out = row-sum(out).
                # Native InstTensorScalarPtr — one DVE pass for the dot.
                nc.vector.scalar_tensor_tensor(
                    out=prod[:],
                    in0=rt[:],
                    scalar=1.0,
                    in1=dt[:],
                    op0=mybir.AluOpType.bypass,
                    op1=mybir.AluOpType.mult,
                    accum_out=acc_dot[:, t : t + 1],
                )
                nc.scalar.activation(
                    rsq[:],
                    rt[:],
                    mybir.ActivationFunctionType.Square,
                    accum_out=acc_rr[:, t : t + 1],
                )
                nc.scalar.activation(
                    dsq[:],
                    dt[:],
                    mybir.ActivationFunctionType.Square,
                    accum_out=acc_dd[:, t : t + 1],
                )
            nc.sync.dma_start(out=o[0], in_=acc_dot[:])
            nc.sync.dma_start(out=o[1], in_=acc_rr[:])
            nc.sync.dma_start(out=o[2], in_=acc_dd[:])
    nc.compile()
    return nc


def _get_nc():
    if "nc" not in _CACHE:
        _CACHE["nc"] = _build()
    return _CACHE["nc"]


def _pack(arrays):
    buf = np.zeros(PADDED, dtype=np.float32)
    off = 0
    for a in arrays:
        a = np.asarray(a, dtype=np.float32)
        n = a.size
        buf[off : off + n] = a.reshape(-1)
        off += n
    assert off == TOTAL
    return buf.reshape(C, T, P, F)


def _run(inputs, trace=False):
    rbuf = _pack([inputs[k] for k in _REC_KEYS])
    dbuf = _pack([inputs[k] for k in _DATA_KEYS])
    in_maps = [{"r": rbuf[c], "d": dbuf[c]} for c in range(C)]
    res = run_bass_kernel_spmd(_get_nc(), in_maps, core_ids=list(range(C)), trace=trace)
    tot = np.zeros(3, dtype=np.float64)
    for m in res.results:
        tot += m["o"].reshape(3, -1).astype(np.float64).sum(axis=1)
    sp, rn, dn = tot
    out = 1.0 - sp / (np.sqrt(rn) * np.sqrt(dn))
    return np.array(out, dtype=np.float32), res


def kernel(**inputs):
    out, _ = _run(inputs, trace=False)
    return out


def kernel_traced(**inputs):
    out, res = _run(inputs, trace=True)
    return out, res


# revision 5
# speedup vs baseline: 1.3678x; 1.1837x over previous
"""Trainium2 Bass kernel for fused multi-tensor cosine-similarity loss.

Computes 1 - <r,d> / (|r| |d|) over 10 gradient tensors (5 rec + 5 data,
45,675,264 f32 elements per side), data-parallel across 8 NeuronCores.

Strategy (memory-bound, ~45.7 MB HBM traffic per core):
  - Host packs each side into a flat f32 stream, zero-padded to
    8 cores x T tiles x 128 partitions x F columns (zeros don't affect
    dot products or squared norms).
  - Per core, per tile: DMA r,d tiles to SBUF; one DVE
    tensor_tensor_reduce computes r*d and its per-partition row-sum in a
    single pass; two ACT activation(Square, accum_out=...) compute the
    per-partition row-sums of r^2 and d^2 in one pass each.
  - Per-tile partial sums land in [128, T] accumulators, DMA'd out once.
  - Host reduces the 8 x 3 x 128 x T partials in float64 and applies the
    final cosine combine.
"""

import os
import sys

import numpy as np

_REPO = "/opt/trn_rl_repo"
if _REPO not in sys.path:
    sys.path.insert(0, _REPO)

import concourse.bacc as bacc
import concourse.mybir as mybir
from concourse.bass_utils import run_bass_kernel_spmd
from concourse.tile import TileContext

C = 8  # cores
P = 128  # SBUF partitions
F = 2048  # free-dim columns per tile (1 MiB per DMA)
T = 22  # tiles per core
TOTAL = 45_675_264  # elements per side (sum of the 5 tensor sizes)
PADDED = C * T * P * F  # 46,137,344

_REC_KEYS = ("rec_emb", "rec_qkv", "rec_proj", "rec_fc1", "rec_fc2")
_DATA_KEYS = ("data_emb", "data_qkv", "data_proj", "data_fc1", "data_fc2")

_CACHE = {}


def _build():
    nc = bacc.Bacc("TRN2", target_bir_lowering=False, debug=False)
    r = nc.declare_dram_parameter("r", [T, P, F], mybir.dt.float32, isOutput=False)
    d = nc.declare_dram_parameter("d", [T, P, F], mybir.dt.float32, isOutput=False)
    o = nc.declare_dram_parameter("o", [3, P, T], mybir.dt.float32, isOutput=True)

    f32 = mybir.dt.float32
    with TileContext(nc) as tc:
        with (
            tc.tile_pool(name="io", bufs=6) as io,
            tc.tile_pool(name="scr", bufs=1) as scr,
            tc.tile_pool(name="accp", bufs=1) as accp,
        ):
            acc_dot = accp.tile([P, T], f32)
            acc_rr = accp.tile([P, T], f32)
            acc_dd = accp.tile([P, T], f32)
            for t in range(T):
                rt = io.tile([P, F], f32, tag="rt")
                dt = io.tile([P, F], f32, tag="dt")
                nc.sync.dma_start(out=rt[:], in_=r[t])
                nc.sync.dma_start(out=dt[:], in_=d[t])
                # Per-engine scratch so DVE and ACT never share a sink tile
                # (cross-engine WAW would serialize them).
                dve_o = scr.tile([P, F], f32, tag="dve_o", bufs=2)
                act_o = scr.tile([P, F], f32, tag="act_o", bufs=2)
                # out = (rt bypass 1.0) * dt; accum_out = row-sum(out).
                # Native InstTensorScalarPtr — one DVE pass for the dot.
                nc.vector.scalar_tensor_tensor(
                    out=dve_o[:],
                    in0=rt[:],
                    scalar=1.0,
                    in1=dt[:],
                    op0=mybir.AluOpType.bypass,
                    op1=mybir.AluOpType.mult,
                    accum_out=acc_dot[:, t : t + 1],
                )
                # Balance the two squares across ACT and DVE: ACT alone
                # (2 squares + accum-reads ~5.3us/tile) can't keep up with
                # DMA (~5.0us/tile-pair at 420 GB/s), so DVE takes r^2 on
                # odd tiles via another STT pass.
                if t % 2 == 0:
                    nc.scalar.activation(
                        act_o[:],
                        rt[:],
                        mybir.ActivationFunctionType.Square,
                        accum_out=acc_rr[:, t : t + 1],
                    )
                else:
                    dve_o2 = scr.tile([P, F], f32, tag="dve_o2", bufs=2)
                    nc.vector.scalar_tensor_tensor(
                        out=dve_o2[:],
                        in0=rt[:],
                        scalar=1.0,
                        in1=rt[:],
                        op0=mybir.AluOpType.bypass,
                        op1=mybir.AluOpType.mult,
                        accum_out=acc_rr[:, t : t + 1],
                    )
                nc.scalar.activation(
                    act_o[:],
                    dt[:],
                    mybir.ActivationFunctionType.Square,
                    accum_out=acc_dd[:, t : t + 1],
                )
            nc.sync.dma_start(out=o[0], in_=acc_dot[:])
            nc.sync.dma_start(out=o[1], in_=acc_rr[:])
            nc.sync.dma_start(out=o[2], in_=acc_dd[:])
    nc.compile()
    return nc


def _get_nc():
    if "nc" not in _CACHE:
        _CACHE["nc"] = _build()
    return _CACHE["nc"]


def _pack(arrays):
    buf = np.zeros(PADDED, dtype=np.float32)
    off = 0
    for a in arrays:
        a = np.asarray(a, dtype=np.float32)
        n = a.size
        buf[off : off + n] = a.reshape(-1)
        off += n
    assert off == TOTAL
    return buf.reshape(C, T, P, F)


def _run(inputs, trace=False):
    rbuf = _pack([inputs[k] for k in _REC_KEYS])
    dbuf = _pack([inputs[k] for k in _DATA_KEYS])
    in_maps = [{"r": rbuf[c], "d": dbuf[c]} for c in range(C)]
    res = run_bass_kernel_spmd(_get_nc(), in_maps, core_ids=list(range(C)), trace=trace)
    tot = np.zeros(3, dtype=np.float64)
    for m in res.results:
        tot += m["o"].reshape(3, -1).astype(np.float64).sum(axis=1)
    sp, rn, dn = tot
    out = 1.0 - sp / (np.sqrt(rn) * np.sqrt(dn))
    return np.array(out, dtype=np.float32), res


def kernel(**inputs):
    out, _ = _run(inputs, trace=False)
    return out


def kernel_traced(**inputs):
    out, res = _run(inputs, trace=True)
    return out, res
